# revision 11
# baseline (speedup 1.0000x reference)
"""2-layer GAT on Trainium2 (8 NeuronCores), self-contained.

Sharding: edges partitioned by dst range (core r owns dst in [r*12500,(r+1)*12500)).
Per layer: node-phase matmul builds per-node tables [h | alpha_src] (+ [alpha_dst]
local), AllGather replicates the src-side table, edge phase dma_gathers per-edge
rows, computes attention with one-hot(dst-offset) matmuls that perform the
segment softmax denominator and the weighted aggregation in the same PSUM
accumulation. Layer 2 ends with head-mean + bias + log_softmax.

Edge layout: chunks are ordered bucket-major within each SUPER-window group so
gather calls merge across windows (fewer SWDGE launches), and the one-hot
builds run once per group in DVE fast-mode-friendly layouts:
  ohT [dst(part), chunk, lane] via tensor_scalar is_equal (per-partition iota
  scalar -> 4x mode), oh [lane(part), dstoff, chunk] via tensor_tensor
  is_equal against a materialized iota table (chunk-innermost -> 2x mode).
Payloads are channel-major in both layers so the attention multiply broadcasts
along the middle (not innermost) axis, keeping 2x DVE mode.
"""
import os
import numpy as np
import ml_dtypes

import concourse.bacc as bacc
import concourse.mybir as mybir
import concourse.tile as tile
from concourse.bass_utils import run_bass_kernel_spmd

BF16 = ml_dtypes.bfloat16

N_NODES = 100000
N_EDGES = 1600000
R = 8
NLOC = N_NODES // R            # 12500
WIN = 128
NWIN = (NLOC + WIN - 1) // WIN  # 98
NEG_SLOPE = 0.2
EPS = 1e-16
NBUCK = 4
SLICE_H = NLOC // NBUCK        # 3125 rows each core contributes per slice
SLAB = R * SLICE_H             # 25000-row gather region, int16-safe
SUPER = int(os.environ.get("K_SUPER", "3"))   # windows per gather group
CALL_MAX = int(os.environ.get("K_CALLMAX", "8"))  # chunks per dma_gather call
DMA_SCRATCH = int(os.environ.get("K_SCRATCH", "16384"))
PAD_DOFF = 20000.0
ZINIT = os.environ.get("K_ZINIT", "1") == "1"
GBUFS = int(os.environ.get("K_GBUFS", "2"))
ABL_NO_PAY = os.environ.get("K_NO_PAY", "0") == "1"
ABL_NO_COLL = os.environ.get("K_NO_COLL", "0") == "1"
ABL_NO_AGG = os.environ.get("K_NO_AGG", "0") == "1"
ABL_NO_ADST = os.environ.get("K_NO_ADST", "0") == "1"
ABL_NO_OH = os.environ.get("K_NO_OH", "0") == "1"
ABL_NO_MULT = os.environ.get("K_NO_MULT", "0") == "1"


def _ceil(a, b):
    return (a + b - 1) // b


def _groups():
    gs = []
    w = 0
    while w < NWIN:
        gs.append((w, min(SUPER, NWIN - w)))
        w += SUPER
    return gs


def _build_layout(src, dst):
    """Static edge layout, uniform across cores. Chunk order: per SUPER-window
    group, bucket-major then window, dst-sorted, in chunks of 128 lanes."""
    groups = _groups()
    core_of = dst // NLOC
    per_core = []
    cnt = np.zeros((R, NWIN, NBUCK), dtype=np.int64)
    for r in range(R):
        sel = np.nonzero(core_of == r)[0]
        s_ = src[sel]
        d_ = dst[sel] - r * NLOC
        w_of = d_ // WIN
        b_of = (s_ % NLOC) // SLICE_H
        np.add.at(cnt[r], (w_of, b_of), 1)
        per_core.append((s_, d_, w_of, b_of))

    nchwb = np.zeros((NWIN, NBUCK), dtype=np.int64)
    for w in range(NWIN):
        for b in range(NBUCK):
            nchwb[w, b] = max(_ceil(int(cnt[r, w, b]), WIN) for r in range(R))

    # group metadata: chunk order within group = (b, w, ci)
    ginfo = []    # (w0, nwg, gc0, nch_g, calls, win_pos, runs=[(c0,c1,b)])
    gc0 = 0
    chunk_window = []
    chunk_cell = []   # (w, b, ci) per global chunk, in order
    for (w0, nwg) in groups:
        nch_g = int(nchwb[w0:w0 + nwg].sum())
        calls = []
        runs = []
        win_pos = {wi: [] for wi in range(w0, w0 + nwg)}
        c = 0
        for b in range(NBUCK):
            seg0 = c
            for wi in range(w0, w0 + nwg):
                for ci in range(int(nchwb[wi, b])):
                    win_pos[wi].append(c)
                    chunk_window.append(wi)
                    chunk_cell.append((wi, b, ci))
                    c += 1
            runs.append((seg0, c, b))
            s = seg0
            while s < c:
                e = min(s + CALL_MAX, c)
                calls.append((s, e, b))
                s = e
        assert c == nch_g
        ginfo.append((w0, nwg, gc0, nch_g, calls, win_pos, runs))
        gc0 += nch_g
    TCH = gc0
    NCHG = max(g[3] for g in ginfo)
    RUNMAX = max(c1 - c0 for g in ginfo for (c0, c1, b) in g[6])

    cores = []
    for r in range(R):
        s_, d_, w_of, b_of = per_core[r]
        order = np.lexsort((d_, w_of, b_of, w_of // SUPER))
        s_, d_, w_of, b_of = s_[order], d_[order], w_of[order], b_of[order]
        # bounds per (group-major sort) cell
        key = (w_of // SUPER) * (NBUCK * NWIN) + b_of * NWIN + w_of
        # edges are sorted by key; find cell boundaries
        srcoff = (np.arange(128 * TCH, dtype=np.int64).reshape(TCH, 128).T
                  * 37) % SLAB
        dstoff = np.full((128, TCH), PAD_DOFF, dtype=np.float32)
        # per-cell fill
        cell_keys = key
        uniq, starts = np.unique(cell_keys, return_index=True)
        bounds = dict(zip(uniq.tolist(), starts.tolist()))
        ends = dict(zip(uniq.tolist(), (np.r_[starts[1:], len(key)]).tolist()))
        gc = 0
        for (w0, nwg, g0, nch_g, calls, win_pos, runs) in ginfo:
            for ci_g in range(nch_g):
                wi, b, ci = chunk_cell[g0 + ci_g]
                k = (wi // SUPER) * (NBUCK * NWIN) + b * NWIN + wi
                lo = bounds.get(k, 0)
                hi = ends.get(k, 0)
                e0 = lo + ci * WIN
                kk = max(0, min(WIN, hi - e0))
                if kk > 0:
                    sv = s_[e0:e0 + kk]
                    srcoff[:kk, g0 + ci_g] = ((sv // NLOC) * SLICE_H
                                              + sv % NLOC - b * SLICE_H)
                    dstoff[:kk, g0 + ci_g] = d_[e0:e0 + kk] - wi * WIN
                gc += 1

        def wrap(a64, ncols):
            a = a64.astype(np.int16)
            w16 = a.T.reshape(ncols, 8, 16).transpose(2, 0, 1).reshape(16, ncols * 8)
            return np.tile(w16, (8, 1))
        dstoff_bf = dstoff.astype(BF16)
        # per-call position of the last real (non-pad) lane, for tail trim
        real = dstoff != PAD_DOFF
        last_real = np.zeros(TCH, dtype=np.int64)
        for c in range(TCH):
            nz = np.nonzero(real[:, c])[0]
            last_real[c] = (nz[-1] + 1) if len(nz) else 0
        cores.append(dict(
            pay_idx=wrap(srcoff, TCH),
            dstoff=np.ascontiguousarray(dstoff_bf),
            dstoffT=np.ascontiguousarray(dstoff_bf.T.reshape(1, TCH * 128)),
            last_real=last_real,
        ))
    # uniform per-call valid count: max over cores of the last real lane
    # position within the call, rounded up to 16
    call_valid = []
    for (w0, nwg, gc0, nch_g, calls, win_pos, runs) in ginfo:
        for (c0, c1, b) in calls:
            k = c1 - c0
            cv = 0
            for cr in cores:
                lr = cr["last_real"][gc0 + c0:gc0 + c1]
                pos = 0
                for j in range(k):
                    if lr[j] > 0:
                        pos = j * 128 + int(lr[j])
                cv = max(cv, pos)
            cv = min(_ceil(cv, 16) * 16, k * 128)
            call_valid.append(k * 128)  # trim disabled: stale-lane fp8 NaN hazard
    return dict(nchwb=nchwb, TCH=TCH, NCHG=NCHG, RUNMAX=RUNMAX, ginfo=ginfo,
                chunk_window=chunk_window, cores=cores, call_valid=call_valid)


def _unified_act_tables(orig):
    """Restrict the act-table chooser to the one set containing every
    function this kernel uses (exp/ln/prelu/relu/copy), so the Activation
    engine never reloads tables mid-kernel."""
    def patched(arch):
        tabs = orig(arch)
        assert "natural_log_exp_and_others" in tabs, list(tabs)
        return {name: (s if name == "natural_log_exp_and_others" else set())
                for name, s in tabs.items()}
    return patched


def build_kernel(lay):
    dt = mybir.dt
    TCH, NCHG, ginfo = lay["TCH"], lay["NCHG"], lay["ginfo"]
    RUNMAX = lay["RUNMAX"]
    call_valid = lay["call_valid"]
    chunk_window = lay["chunk_window"]
    nc = bacc.Bacc(None, target_bir_lowering=False, debug=True,
                   num_swdge_queues=4, dynamic_dma_scratch_size=DMA_SCRATCH)

    xT = nc.dram_tensor("xT", [128, NLOC], dt.float32r, kind="ExternalInput")
    w1p = nc.dram_tensor("w1p", [128, 256], dt.float32r, kind="ExternalInput")
    w2p = nc.dram_tensor("w2p", [64, 256], dt.float32r, kind="ExternalInput")
    b1rep = nc.dram_tensor("b1rep", [128, 64], dt.float32, kind="ExternalInput")
    b2rep = nc.dram_tensor("b2rep", [128, 20], dt.float32, kind="ExternalInput")
    ident = nc.dram_tensor("ident", [128, 128], dt.float32, kind="ExternalInput")
    doff_in = nc.dram_tensor("doff", [128, TCH], dt.bfloat16, kind="ExternalInput")
    doffT_in = nc.dram_tensor("doffT", [1, TCH * 128], dt.bfloat16, kind="ExternalInput")
    iotc_in = nc.dram_tensor("iotc", [128, 128 * NCHG], dt.bfloat16, kind="ExternalInput")
    iotP_in = nc.dram_tensor("iotP", [128, 1], dt.float32, kind="ExternalInput")
    pay_idx = nc.dram_tensor("pay_idx", [128, TCH * 8], dt.int16, kind="ExternalInput")
    # partition-major staging: row (w*128+p) of the logical [NLOC, 20] output
    # lives at [p, w*20:(w+1)*20]; host permutes.
    out_ext = nc.dram_tensor("out", [128, NWIN * 20], dt.float32, kind="ExternalOutput")

    ts1_loc = nc.dram_tensor("ts1_loc", [NLOC, 128], dt.bfloat16)
    ts1_full = nc.dram_tensor("ts1_full", [N_NODES, 128], dt.bfloat16, addr_space="Shared")
    ts2_loc = nc.dram_tensor("ts2_loc", [NLOC, 256], dt.int8)
    ts2_full = nc.dram_tensor("ts2_full", [N_NODES, 256], dt.int8, addr_space="Shared")

    with tile.TileContext(nc) as tc:
        with (
            tc.tile_pool(name="const", bufs=1) as cpool,
            tc.tile_pool(name="sb", bufs=2) as sb,
            tc.tile_pool(name="gb", bufs=GBUFS) as gb,
            tc.tile_pool(name="ps", bufs=2, space="PSUM") as ps,
        ):
            w1t = cpool.tile([128, 256], dt.float32r, name="w1t")
            nc.sync.dma_start(out=w1t[:], in_=w1p[:])
            w2t = cpool.tile([64, 256], dt.float32r, name="w2t")
            nc.sync.dma_start(out=w2t[:], in_=w2p[:])
            b1t = cpool.tile([128, 64], dt.float32, name="b1t")
            nc.sync.dma_start(out=b1t[:], in_=b1rep[:])
            b2t = cpool.tile([128, 20], dt.float32, name="b2t")
            nc.sync.dma_start(out=b2t[:], in_=b2rep[:])
            idt = cpool.tile([128, 128], dt.float32, name="idt")
            nc.sync.dma_start(out=idt[:], in_=ident[:])
            dofft = cpool.tile([128, TCH], dt.bfloat16, name="dofft")
            nc.sync.dma_start(out=dofft[:], in_=doff_in[:])
            iotc = cpool.tile([128, 128, NCHG], dt.bfloat16, name="iotc")
            nc.sync.dma_start(out=iotc[:], in_=iotc_in[:].rearrange(
                "p (j c) -> p j c", c=NCHG))
            iotP = cpool.tile([128, 1], dt.float32, name="iotP")
            nc.sync.dma_start(out=iotP[:], in_=iotP_in[:])
            # per-window [node, head] attention-dst tables, SBUF-resident
            tdT1 = cpool.tile([128, NWIN * 8], dt.bfloat16, name="tdT1")
            tdT2 = cpool.tile([128, NWIN * 8], dt.bfloat16, name="tdT2")
            nc.scalar.memzero(tdT1[:])
            nc.scalar.memzero(tdT2[:])
            outsb = cpool.tile([128, NWIN * 20], dt.float32, name="outsb")
            nc.scalar.memzero(outsb[:])

            # ---------- Phase A: T1 rows for local nodes ----------
            PHB = 4  # windows per xT load
            xt = None
            for w in range(NWIN):
                nw = min(WIN, NLOC - w * WIN)
                if w % PHB == 0:
                    ncols = min(PHB * WIN, NLOC - w * WIN)
                    xt = sb.tile([128, PHB * WIN], dt.float32r,
                                 name=f"xt{w}", tag="xt", bufs=3)
                    nc.sync.dma_start(out=xt[:, :ncols],
                                      in_=xT[:, w * WIN:w * WIN + ncols])
                xo = (w % PHB) * WIN
                pA = ps.tile([WIN, 256], dt.float32, name=f"pA{w}", tag="pA")
                nc.tensor.matmul(out=pA[:nw, 0:80], lhsT=xt[:, xo:xo + nw],
                                 rhs=w1t[:, 0:80], start=True, stop=True)
                t1 = sb.tile([WIN, 80], dt.bfloat16, name=f"t1o{w}", tag="t1o", bufs=3)
                nc.vector.tensor_copy(out=t1[:nw, :], in_=pA[:nw, 0:80])
                nc.sync.dma_start(out=ts1_loc[w * WIN:w * WIN + nw, 0:72],
                                  in_=t1[:nw, 0:72])
                nc.vector.tensor_copy(out=tdT1[:nw, w * 8:w * 8 + 8],
                                      in_=t1[:nw, 72:80])

            if not ABL_NO_COLL:
                for s in range(NBUCK):
                    nc.gpsimd.collective_compute(
                        "AllGather", mybir.AluOpType.bypass,
                        replica_groups=[list(range(R))],
                        ins=[ts1_loc[s * SLICE_H:(s + 1) * SLICE_H]],
                        outs=[ts1_full[s * SLAB:(s + 1) * SLAB]])

            # ---------- window epilogues ----------
            def epilogue1(wi, nw, pagg):
                # channel-major (e, h) layout throughout
                t8 = sb.tile([WIN, 8], dt.float32, name=f"t8a{wi}", tag="t8a")
                nc.vector.tensor_scalar(t8[:nw], pagg[:nw, 64:72], EPS, None,
                                        mybir.AluOpType.add)
                inv8 = sb.tile([WIN, 8], dt.float32, name=f"i8a{wi}", tag="i8a")
                nc.vector.reciprocal(inv8[:nw], t8[:nw])
                z = sb.tile([WIN, 64], dt.float32, name=f"z{wi}", tag="z")
                nc.vector.tensor_tensor(
                    out=z[:nw].rearrange("p (e h) -> p e h", h=8),
                    in0=pagg[:nw, 0:64].rearrange("p (e h) -> p e h", h=8),
                    in1=inv8[:nw].unsqueeze(1).broadcast_to([nw, 8, 8]),
                    op=mybir.AluOpType.mult)
                nc.vector.tensor_tensor(out=z[:nw], in0=z[:nw], in1=b1t[:nw],
                                        op=mybir.AluOpType.add)
                mz = sb.tile([WIN, 64], dt.float32, name=f"mz{wi}", tag="mz")
                nc.vector.tensor_scalar(mz[:nw], z[:nw], 0.0, None,
                                        mybir.AluOpType.min)
                nc.scalar.activation(mz[:nw], mz[:nw],
                                     mybir.ActivationFunctionType.Exp)
                rz = sb.tile([WIN, 64], dt.float32, name=f"rz{wi}", tag="rz")
                nc.scalar.activation(rz[:nw], z[:nw],
                                     mybir.ActivationFunctionType.Relu)
                nc.vector.tensor_tensor(out=z[:nw], in0=mz[:nw], in1=rz[:nw],
                                        op=mybir.AluOpType.add)
                h2 = sb.tile([WIN, 64], dt.float32, name=f"h2{wi}", tag="h2")
                nc.vector.tensor_scalar(h2[:nw], z[:nw], -1.0, None,
                                        mybir.AluOpType.add)
                ptr = ps.tile([64, WIN], dt.float32, name=f"ptr{wi}", tag="ptr")
                nc.tensor.transpose(out=ptr[:, :nw], in_=h2[:nw, :], identity=idt[:nw, :nw])
                h2T = sb.tile([64, WIN], dt.float32r, name=f"h2T{wi}", tag="h2T")
                nc.scalar.copy(out=h2T[:, :nw], in_=ptr[:, :nw])
                pT2 = ps.tile([WIN, 256], dt.float32, name=f"pT2{wi}", tag="pA")
                nc.tensor.matmul(out=pT2[:nw, 0:176], lhsT=h2T[:, :nw],
                                 rhs=w2t[:, 0:176], start=True, stop=True)
                t2p = sb.tile([WIN, 176], dt.int8, name=f"t2p{wi}", tag="t2p", bufs=3)
                nc.scalar.copy(out=t2p[:nw, 0:160].bitcast(dt.float8e4),
                               in_=pT2[:nw, 0:160])
                nc.vector.tensor_copy(out=t2p[:nw, 160:176].bitcast(dt.bfloat16),
                                      in_=pT2[:nw, 160:168])
                nc.sync.dma_start(out=ts2_loc[wi * WIN:wi * WIN + nw, 0:176],
                                  in_=t2p[:nw, 0:176])
                nc.vector.tensor_copy(out=tdT2[:nw, wi * 8:wi * 8 + 8],
                                      in_=pT2[:nw, 168:176])

            def epilogue2(wi, nw, pagg):
                t8 = sb.tile([WIN, 8], dt.float32, name=f"t8b{wi}", tag="t8b")
                nc.vector.tensor_scalar(t8[:nw], pagg[:nw, 160:168], EPS, 8.0,
                                        mybir.AluOpType.add, mybir.AluOpType.mult)
                inv8 = sb.tile([WIN, 8], dt.float32, name=f"i8b{wi}", tag="i8b")
                nc.vector.reciprocal(inv8[:nw], t8[:nw])
                v = sb.tile([WIN, 20], dt.float32, name=f"v{wi}", tag="v")
                val = sb.tile([WIN, 160], dt.float32, name=f"val{wi}", tag="val")
                nc.vector.tensor_tensor(
                    out=val[:nw].rearrange("p (c h) -> p c h", h=8),
                    in0=pagg[:nw, 0:160].rearrange("p (c h) -> p c h", h=8),
                    in1=inv8[:nw].unsqueeze(1).broadcast_to([nw, 20, 8]),
                    op=mybir.AluOpType.mult)
                nc.vector.tensor_reduce(
                    out=v[:nw], in_=val[:nw].rearrange("p (c h) -> p c h", h=8),
                    axis=mybir.AxisListType.X, op=mybir.AluOpType.add)
                nc.vector.tensor_tensor(out=v[:nw], in0=v[:nw], in1=b2t[:nw],
                                        op=mybir.AluOpType.add)
                mx = sb.tile([WIN, 1], dt.float32, name=f"mx{wi}", tag="mx")
                nc.vector.tensor_reduce(out=mx[:nw], in_=v[:nw],
                                        axis=mybir.AxisListType.X,
                                        op=mybir.AluOpType.max)
                nmx = sb.tile([WIN, 1], dt.float32, name=f"nmx{wi}", tag="nmx")
                nc.vector.tensor_scalar(nmx[:nw], mx[:nw], -1.0, None,
                                        mybir.AluOpType.mult)
                ex = sb.tile([WIN, 20], dt.float32, name=f"exo{wi}", tag="exo")
                ssum = sb.tile([WIN, 1], dt.float32, name=f"ss{wi}", tag="ss")
                nc.scalar.activation(ex[:nw], v[:nw],
                                     mybir.ActivationFunctionType.Exp,
                                     bias=nmx[:nw], accum_out=ssum[:nw])
                lse = sb.tile([WIN, 1], dt.float32, name=f"lse{wi}", tag="lse")
                nc.scalar.activation(lse[:nw], ssum[:nw],
                                     mybir.ActivationFunctionType.Ln)
                nc.vector.tensor_tensor(
                    out=ex[:nw], in0=v[:nw],
                    in1=nmx[:nw].broadcast_to([nw, 20]),
                    op=mybir.AluOpType.add)
                nc.vector.tensor_tensor(
                    out=outsb[:nw, wi * 20:wi * 20 + 20], in0=ex[:nw],
                    in1=lse[:nw].broadcast_to([nw, 20]),
                    op=mybir.AluOpType.subtract)

            # ---------- slot init: zero gbuf pool slots so pad lanes read
            # finite stale data ----------
            if ZINIT:
                for ii in range(GBUFS):
                    gz = gb.tile([128, NCHG, 256], dt.bfloat16,
                                 name=f"gz{ii}", tag="gbuf", bufs=GBUFS)
                    nc.scalar.memzero(gz[:])

            # ---------- edge phase ----------
            def edge_phase(layer, slice_done=None):
                # last window whose epilogue completes collective slice s
                slice_last_w = [(min((s + 1) * SLICE_H, NLOC) - 1) // WIN
                                for s in range(NBUCK)]
                if layer == 1:
                    tsrc_full, tdT = ts1_full, tdT1
                    EW, CY, CH = 128, 72, 64
                    gdt = dt.bfloat16
                else:
                    tsrc_full, tdT = ts2_full, tdT2
                    EW, CY, CH = 256, 168, 160
                    gdt = dt.int8

                qn = 0
                call_idx = [0]
                for (w0, nwg, gc0, nch_g, calls, win_pos, runs) in ginfo:
                    gbuf = gb.tile([128, nch_g, EW], gdt,
                                   name=f"gbuf{layer}_{w0}", tag="gbuf", bufs=GBUFS,
                                   padded_shape=[128, NCHG * 2, EW])
                    pidxg = sb.tile([128, nch_g * 8], dt.int16,
                                    name=f"pi{layer}_{w0}", tag="pidx",
                                    padded_shape=[128, NCHG * 8])
                    nc.sync.dma_start(out=pidxg[:],
                                      in_=pay_idx[:, gc0 * 8:(gc0 + nch_g) * 8])
                    for (c0, c1, b) in calls:
                        k = c1 - c0
                        cv = call_valid[call_idx[0]]
                        call_idx[0] += 1
                        if ABL_NO_PAY:
                            continue
                        nc.gpsimd.dma_gather(
                            gbuf[:, c0:c1, :],
                            tsrc_full[b * SLAB:(b + 1) * SLAB, :],
                            pidxg[:, c0 * 8:c1 * 8],
                            k * 128, cv, EW,
                            queue_num=qn % 4, single_packet=True)
                        qn += 1
                    # dst-side attention: ohT [dst(part), chunk, lane],
                    # built per bucket-run to bound SBUF
                    eadst = ps.tile([128, nch_g * 8], dt.float32,
                                    name=f"ea{layer}_{w0}", tag="eadst",
                                    padded_shape=[128, NCHG * 8])
                    for (c0, c1, b) in (runs if not ABL_NO_ADST else []):
                        rl = c1 - c0
                        dfTg = sb.tile([128, rl, 128], dt.bfloat16,
                                       name=f"dfT{layer}_{w0}_{b}", tag="dfT",
                                       padded_shape=[128, RUNMAX, 128])
                        nc.sync.dma_start(
                            out=dfTg[:],
                            in_=doffT_in[0:1, (gc0 + c0) * 128:(gc0 + c1) * 128]
                                .broadcast_to([128, rl * 128])
                                .rearrange("p (c l) -> p c l", l=128))
                        ohTg = sb.tile([128, rl, 128], dt.bfloat16,
                                       name=f"ohT{layer}_{w0}_{b}", tag="ohT",
                                       padded_shape=[128, RUNMAX, 128])
                        nc.vector.tensor_scalar(ohTg[:], dfTg[:], iotP[:, 0:1],
                                                None, mybir.AluOpType.is_equal)
                        for ci in range(c0, c1):
                            wi = chunk_window[gc0 + ci]
                            nc.tensor.matmul(
                                out=eadst[:, ci * 8:ci * 8 + 8],
                                lhsT=ohTg[:, ci - c0, :],
                                rhs=tdT[:, wi * 8:wi * 8 + 8],
                                start=True, stop=True)
                    ebuf = sb.tile([128, nch_g, 8], dt.float32,
                                   name=f"eb{layer}_{w0}", tag="ebuf",
                                   padded_shape=[128, NCHG, 8])
                    asrc_ap = (gbuf[:, :, CH:CH + 8] if layer == 1 else
                               gbuf[:, :, 160:176].bitcast(dt.bfloat16))
                    if ABL_NO_ADST:
                        nc.vector.tensor_copy(out=ebuf[:], in_=asrc_ap)
                    else:
                        nc.vector.tensor_tensor(
                            out=ebuf[:], in0=asrc_ap,
                            in1=eadst[:].rearrange("p (c h) -> p c h", h=8),
                            op=mybir.AluOpType.add)
                    nc.scalar.activation(ebuf[:], ebuf[:],
                                         mybir.ActivationFunctionType.Prelu,
                                         alpha=NEG_SLOPE)
                    ne = CH // 8
                    if layer == 1:
                        agg_src = gbuf
                        nc.scalar.activation(gbuf[:, :, CH:CH + 8], ebuf[:],
                                             mybir.ActivationFunctionType.Exp)
                    else:
                        # bf16 working copy: fp8->bf16 payload (Act) + exp tail
                        wp = sb.tile([128, nch_g, 168], dt.bfloat16,
                                     name=f"wp{w0}", tag="wp",
                                     padded_shape=[128, NCHG, 168])
                        nc.scalar.copy(out=wp[:, :, 0:160],
                                       in_=gbuf[:, :, 0:160].bitcast(dt.float8e4))
                        nc.scalar.activation(wp[:, :, 160:168], ebuf[:],
                                             mybir.ActivationFunctionType.Exp)
                        agg_src = wp
                    # channel-major payload: (e-or-c, h) with h innermost
                    if not ABL_NO_MULT:
                        nc.vector.tensor_tensor(
                            out=agg_src[:, :, 0:CH].rearrange(
                                "p c (e h) -> p c e h", h=8),
                            in0=agg_src[:, :, 0:CH].rearrange(
                                "p c (e h) -> p c e h", h=8),
                            in1=agg_src[:, :, CH:CH + 8].unsqueeze(2)
                                .broadcast_to([128, nch_g, ne, 8]),
                            op=mybir.AluOpType.mult)
                    # aggregation one-hot oh [lane(part), dstoff, chunk]
                    ohg = sb.tile([128, 128, nch_g], dt.bfloat16,
                                  name=f"oh{layer}_{w0}", tag="oh",
                                  padded_shape=[128, 128, NCHG])
                    if not ABL_NO_OH:
                        nc.vector.tensor_tensor(
                            out=ohg[:],
                            in0=dofft[:, gc0:gc0 + nch_g].unsqueeze(1)
                                .broadcast_to([128, 128, nch_g]),
                            in1=iotc[:, :, :nch_g],
                            op=mybir.AluOpType.is_equal)
                    for wi in range(w0, w0 + nwg):
                        nw_nodes = min(WIN, NLOC - wi * WIN)
                        pos = win_pos[wi]
                        pagg = ps.tile([WIN, CY], dt.float32,
                                       name=f"pg{layer}_{wi}", tag="pagg",
                                       padded_shape=[WIN, 168])
                        nmm = 1 if ABL_NO_AGG else len(pos)
                        for j in range(nmm):
                            ci = pos[j]
                            nc.tensor.matmul(
                                out=pagg[:, :],
                                lhsT=ohg[:, :, ci],
                                rhs=agg_src[:, ci, 0:CY],
                                start=(j == 0), stop=(j == nmm - 1))
                        if layer == 1:
                            epilogue1(wi, nw_nodes, pagg)
                        else:
                            epilogue2(wi, nw_nodes, pagg)
                        if slice_done is not None and wi in slice_last_w:
                            slice_done(slice_last_w.index(wi))

            def coll2_slice(s):
                if not ABL_NO_COLL:
                    nc.gpsimd.collective_compute(
                        "AllGather", mybir.AluOpType.bypass,
                        replica_groups=[list(range(R))],
                        ins=[ts2_loc[s * SLICE_H:(s + 1) * SLICE_H]],
                        outs=[ts2_full[s * SLAB:(s + 1) * SLAB]])

            edge_phase(1, coll2_slice)
            edge_phase(2)
            nc.sync.dma_start(out=out_ext[:], in_=outsb[:])

    orig = bacc.get_activation_tables
    bacc.get_activation_tables = _unified_act_tables(orig)
    try:
        nc.compile()
    finally:
        bacc.get_activation_tables = orig
    return nc


def _wext(w, a_src, a_dst, out_rows, pad_cols=256, heads=8):
    """[W(ch-major) | W.a_src | W.a_dst] padded to [out_rows, pad_cols] f32.
    Output feature columns are channel-major: col = e*H + h."""
    f = w.shape[0]
    c = w.shape[1] // heads
    w3 = w.reshape(f, heads, c)
    wmain = w3.transpose(0, 2, 1).reshape(f, heads * c)  # col = e*H + h
    was = np.einsum("fhc,hc->fh", w3, a_src)
    wad = np.einsum("fhc,hc->fh", w3, a_dst)
    out = np.zeros((out_rows, pad_cols), dtype=np.float32)
    out[:f, :w.shape[1]] = wmain
    out[:f, w.shape[1]:w.shape[1] + heads] = was
    out[:f, w.shape[1] + heads:w.shape[1] + 2 * heads] = wad
    return out


_CACHE = {}


def kernel(x, edge_index, w1, att_src1, att_dst1, b1, w2, att_src2, att_dst2, b2):
    x = np.asarray(x, dtype=np.float32)
    edge_index = np.asarray(edge_index)
    src = np.concatenate([edge_index[0], np.arange(N_NODES, dtype=np.int64)]).astype(np.int64)
    dst = np.concatenate([edge_index[1], np.arange(N_NODES, dtype=np.int64)]).astype(np.int64)

    key = hash(edge_index.tobytes())
    if key not in _CACHE:
        lay = _build_layout(src, dst)
        nkern = build_kernel(lay)
        _CACHE[key] = (lay, nkern)
    lay, nkern = _CACHE[key]

    in_maps = _prep_in_maps(x, w1, att_src1, att_dst1, b1,
                            w2, att_src2, att_dst2, b2, lay)
    res = run_bass_kernel_spmd(nkern, in_maps, core_ids=list(range(R)))
    parts = []
    for r in range(R):
        o = np.asarray(res.results[r]["out"])  # [128, NWIN*20] partition-major
        o = o.reshape(128, NWIN, 20).transpose(1, 0, 2).reshape(NWIN * 128, 20)
        parts.append(o[:NLOC])
    return np.concatenate(parts, axis=0).astype(np.float32)


def _prep_in_maps(x, w1, att_src1, att_dst1, b1, w2, att_src2, att_dst2, b2, lay):
    NCHG = lay["NCHG"]
    cores = lay["cores"]
    H = 8
    # layer-1 channel-major: h1 columns (e,h); b1 likewise
    w1p = _wext(np.asarray(w1, np.float32), np.asarray(att_src1, np.float32),
                np.asarray(att_dst1, np.float32), 128)
    # layer-2 input rows follow layer-1 ch-major order: permute rows of w2
    w2a = np.asarray(w2, np.float32)
    rowperm = (np.arange(64).reshape(8, 8).T.reshape(64))  # new r=(e*8+h) -> old h*8+e
    w2perm = w2a[rowperm]
    w2p = _wext(w2perm, np.asarray(att_src2, np.float32),
                np.asarray(att_dst2, np.float32), 64)
    b1cm = np.asarray(b1, np.float32).reshape(H, 8).T.reshape(64)
    b1rep = np.tile(b1cm[None, :], (128, 1))
    b2rep = np.tile(np.asarray(b2, np.float32)[None, :], (128, 1))
    ident = np.eye(128, dtype=np.float32)
    iotc = np.broadcast_to(np.arange(128, dtype=np.float32)[:, None],
                           (128, NCHG)).astype(BF16)
    iotc = np.broadcast_to(iotc.reshape(1, 128, NCHG), (128, 128, NCHG))
    iotc = np.ascontiguousarray(iotc.reshape(128, 128 * NCHG))
    iotP = np.arange(128, dtype=np.float32).reshape(128, 1)
    in_maps = []
    for r in range(R):
        xTr = np.ascontiguousarray(x[r * NLOC:(r + 1) * NLOC].T)
        in_maps.append({
            "xT": xTr, "w1p": w1p, "w2p": w2p, "b1rep": b1rep, "b2rep": b2rep,
            "ident": ident,
            "doff": cores[r]["dstoff"], "doffT": cores[r]["dstoffT"],
            "iotc": iotc, "iotP": iotP, "pay_idx": cores[r]["pay_idx"],
        })
    return in_maps


# revision 14
# speedup vs baseline: 1.0930x; 1.0930x over previous
"""2-layer GAT on Trainium2 (8 NeuronCores), self-contained.

Sharding: edges partitioned by dst range (core r owns dst in [r*12500,(r+1)*12500)).
Per layer: node-phase matmul builds per-node tables [h | alpha_src] (+ [alpha_dst]
local), AllGather replicates the src-side table, edge phase dma_gathers per-edge
rows, computes attention with one-hot(dst-offset) matmuls that perform the
segment softmax denominator and the weighted aggregation in the same PSUM
accumulation. Layer 2 ends with head-mean + bias + log_softmax.

Edge layout: chunks are ordered bucket-major within each SUPER-window group so
gather calls merge across windows (fewer SWDGE launches), and the one-hot
builds run once per group in DVE fast-mode-friendly layouts:
  ohT [dst(part), chunk, lane] via tensor_scalar is_equal (per-partition iota
  scalar -> 4x mode), oh [lane(part), dstoff, chunk] via tensor_tensor
  is_equal against a materialized iota table (chunk-innermost -> 2x mode).
Payloads are channel-major in both layers so the attention multiply broadcasts
along the middle (not innermost) axis, keeping 2x DVE mode.
"""
import os
import numpy as np
import ml_dtypes

import concourse.bacc as bacc
import concourse.mybir as mybir
import concourse.tile as tile
from concourse.bass_utils import run_bass_kernel_spmd

BF16 = ml_dtypes.bfloat16

N_NODES = 100000
N_EDGES = 1600000
R = 8
NLOC = N_NODES // R            # 12500
WIN = 128
NWIN = (NLOC + WIN - 1) // WIN  # 98
NEG_SLOPE = 0.2
EPS = 1e-16
NBUCK = 4
SLICE_H = NLOC // NBUCK        # 3125 rows each core contributes per slice
SLAB = R * SLICE_H             # 25000-row gather region, int16-safe
SUPER = int(os.environ.get("K_SUPER", "3"))   # windows per gather group
CALL_MAX = int(os.environ.get("K_CALLMAX", "8"))  # chunks per dma_gather call
DMA_SCRATCH = int(os.environ.get("K_SCRATCH", "16384"))
PAD_DOFF = 20000.0
ZINIT = os.environ.get("K_ZINIT", "1") == "1"
GBUFS = int(os.environ.get("K_GBUFS", "2"))
ABL_NO_PAY = os.environ.get("K_NO_PAY", "0") == "1"
ABL_NO_COLL = os.environ.get("K_NO_COLL", "0") == "1"
ABL_NO_AGG = os.environ.get("K_NO_AGG", "0") == "1"


def _ceil(a, b):
    return (a + b - 1) // b


def _groups():
    gs = []
    w = 0
    while w < NWIN:
        gs.append((w, min(SUPER, NWIN - w)))
        w += SUPER
    return gs


def _build_layout(src, dst):
    """Static edge layout, uniform across cores. Chunk order: per SUPER-window
    group, bucket-major then window, dst-sorted, in chunks of 128 lanes."""
    groups = _groups()
    core_of = dst // NLOC
    per_core = []
    cnt = np.zeros((R, NWIN, NBUCK), dtype=np.int64)
    for r in range(R):
        sel = np.nonzero(core_of == r)[0]
        s_ = src[sel]
        d_ = dst[sel] - r * NLOC
        w_of = d_ // WIN
        b_of = (s_ % NLOC) // SLICE_H
        np.add.at(cnt[r], (w_of, b_of), 1)
        per_core.append((s_, d_, w_of, b_of))

    nchwb = np.zeros((NWIN, NBUCK), dtype=np.int64)
    for w in range(NWIN):
        for b in range(NBUCK):
            nchwb[w, b] = max(_ceil(int(cnt[r, w, b]), WIN) for r in range(R))

    # group metadata: chunk order within group = (b, w, ci)
    ginfo = []    # (w0, nwg, gc0, nch_g, calls, win_pos, runs=[(c0,c1,b)])
    gc0 = 0
    chunk_window = []
    chunk_cell = []   # (w, b, ci) per global chunk, in order
    for (w0, nwg) in groups:
        nch_g = int(nchwb[w0:w0 + nwg].sum())
        calls = []
        runs = []
        win_pos = {wi: [] for wi in range(w0, w0 + nwg)}
        c = 0
        for b in range(NBUCK):
            seg0 = c
            for wi in range(w0, w0 + nwg):
                for ci in range(int(nchwb[wi, b])):
                    win_pos[wi].append(c)
                    chunk_window.append(wi)
                    chunk_cell.append((wi, b, ci))
                    c += 1
            runs.append((seg0, c, b))
            s = seg0
            while s < c:
                e = min(s + CALL_MAX, c)
                calls.append((s, e, b))
                s = e
        assert c == nch_g
        ginfo.append((w0, nwg, gc0, nch_g, calls, win_pos, runs))
        gc0 += nch_g
    TCH = gc0
    NCHG = max(g[3] for g in ginfo)
    RUNMAX = max(c1 - c0 for g in ginfo for (c0, c1, b) in g[6])

    cores = []
    for r in range(R):
        s_, d_, w_of, b_of = per_core[r]
        order = np.lexsort((d_, w_of, b_of, w_of // SUPER))
        s_, d_, w_of, b_of = s_[order], d_[order], w_of[order], b_of[order]
        # bounds per (group-major sort) cell
        key = (w_of // SUPER) * (NBUCK * NWIN) + b_of * NWIN + w_of
        # edges are sorted by key; find cell boundaries
        srcoff = (np.arange(128 * TCH, dtype=np.int64).reshape(TCH, 128).T
                  * 37) % SLAB
        dstoff = np.full((128, TCH), PAD_DOFF, dtype=np.float32)
        # per-cell fill
        cell_keys = key
        uniq, starts = np.unique(cell_keys, return_index=True)
        bounds = dict(zip(uniq.tolist(), starts.tolist()))
        ends = dict(zip(uniq.tolist(), (np.r_[starts[1:], len(key)]).tolist()))
        gc = 0
        for (w0, nwg, g0, nch_g, calls, win_pos, runs) in ginfo:
            for ci_g in range(nch_g):
                wi, b, ci = chunk_cell[g0 + ci_g]
                k = (wi // SUPER) * (NBUCK * NWIN) + b * NWIN + wi
                lo = bounds.get(k, 0)
                hi = ends.get(k, 0)
                e0 = lo + ci * WIN
                kk = max(0, min(WIN, hi - e0))
                if kk > 0:
                    sv = s_[e0:e0 + kk]
                    srcoff[:kk, g0 + ci_g] = ((sv // NLOC) * SLICE_H
                                              + sv % NLOC - b * SLICE_H)
                    dstoff[:kk, g0 + ci_g] = d_[e0:e0 + kk] - wi * WIN
                gc += 1

        def wrap(a64, ncols):
            a = a64.astype(np.int16)
            w16 = a.T.reshape(ncols, 8, 16).transpose(2, 0, 1).reshape(16, ncols * 8)
            return np.tile(w16, (8, 1))
        dstoff_bf = dstoff.astype(BF16)
        cores.append(dict(
            pay_idx=wrap(srcoff, TCH),
            dstoff=np.ascontiguousarray(dstoff_bf),
            dstoffT=np.ascontiguousarray(dstoff_bf.T.reshape(1, TCH * 128)),
        ))
    return dict(nchwb=nchwb, TCH=TCH, NCHG=NCHG, RUNMAX=RUNMAX, ginfo=ginfo,
                chunk_window=chunk_window, cores=cores)


def _unified_act_tables(orig):
    """Restrict the act-table chooser to the one set containing every
    function this kernel uses (exp/ln/prelu/relu/copy), so the Activation
    engine never reloads tables mid-kernel."""
    def patched(arch):
        tabs = orig(arch)
        assert "natural_log_exp_and_others" in tabs, list(tabs)
        return {name: (s if name == "natural_log_exp_and_others" else set())
                for name, s in tabs.items()}
    return patched


def build_kernel(lay):
    dt = mybir.dt
    TCH, NCHG, ginfo = lay["TCH"], lay["NCHG"], lay["ginfo"]
    RUNMAX = lay["RUNMAX"]
    chunk_window = lay["chunk_window"]
    nc = bacc.Bacc(None, target_bir_lowering=False, debug=True,
                   num_swdge_queues=4, dynamic_dma_scratch_size=DMA_SCRATCH,
                   use_seq_codegen=os.environ.get("K_SEQCG", "0") == "1")

    xT = nc.dram_tensor("xT", [128, NLOC], dt.float32r, kind="ExternalInput")
    w1p = nc.dram_tensor("w1p", [128, 256], dt.float32r, kind="ExternalInput")
    w2p = nc.dram_tensor("w2p", [64, 256], dt.float32r, kind="ExternalInput")
    b1rep = nc.dram_tensor("b1rep", [128, 64], dt.float32, kind="ExternalInput")
    b2rep = nc.dram_tensor("b2rep", [128, 20], dt.float32, kind="ExternalInput")
    ident = nc.dram_tensor("ident", [128, 128], dt.float32, kind="ExternalInput")
    doff_in = nc.dram_tensor("doff", [128, TCH], dt.bfloat16, kind="ExternalInput")
    doffT_in = nc.dram_tensor("doffT", [1, TCH * 128], dt.bfloat16, kind="ExternalInput")
    iotc_in = nc.dram_tensor("iotc", [128, 128 * NCHG], dt.bfloat16, kind="ExternalInput")
    iotP_in = nc.dram_tensor("iotP", [128, 1], dt.float32, kind="ExternalInput")
    pay_idx = nc.dram_tensor("pay_idx", [128, TCH * 8], dt.int16, kind="ExternalInput")
    # partition-major staging: row (w*128+p) of the logical [NLOC, 20] output
    # lives at [p, w*20:(w+1)*20]; host permutes.
    out_ext = nc.dram_tensor("out", [128, NWIN * 20], dt.float32, kind="ExternalOutput")

    ts1_loc = nc.dram_tensor("ts1_loc", [NLOC, 128], dt.bfloat16)
    ts1_full = nc.dram_tensor("ts1_full", [N_NODES, 128], dt.bfloat16, addr_space="Shared")
    ts2_loc = nc.dram_tensor("ts2_loc", [NLOC, 256], dt.bfloat16)
    ts2_full = nc.dram_tensor("ts2_full", [N_NODES, 256], dt.bfloat16, addr_space="Shared")

    with tile.TileContext(nc) as tc:
        with (
            tc.tile_pool(name="const", bufs=1) as cpool,
            tc.tile_pool(name="sb", bufs=2) as sb,
            tc.tile_pool(name="gb", bufs=GBUFS) as gb,
            tc.tile_pool(name="ps", bufs=2, space="PSUM") as ps,
        ):
            w1t = cpool.tile([128, 256], dt.float32r, name="w1t")
            nc.sync.dma_start(out=w1t[:], in_=w1p[:])
            w2t = cpool.tile([64, 256], dt.float32r, name="w2t")
            nc.sync.dma_start(out=w2t[:], in_=w2p[:])
            b1t = cpool.tile([128, 64], dt.float32, name="b1t")
            nc.sync.dma_start(out=b1t[:], in_=b1rep[:])
            b2t = cpool.tile([128, 20], dt.float32, name="b2t")
            nc.sync.dma_start(out=b2t[:], in_=b2rep[:])
            idt = cpool.tile([128, 128], dt.float32, name="idt")
            nc.sync.dma_start(out=idt[:], in_=ident[:])
            dofft = cpool.tile([128, TCH], dt.bfloat16, name="dofft")
            nc.sync.dma_start(out=dofft[:], in_=doff_in[:])
            iotc = cpool.tile([128, 128, NCHG], dt.bfloat16, name="iotc")
            nc.sync.dma_start(out=iotc[:], in_=iotc_in[:].rearrange(
                "p (j c) -> p j c", c=NCHG))
            iotP = cpool.tile([128, 1], dt.float32, name="iotP")
            nc.sync.dma_start(out=iotP[:], in_=iotP_in[:])
            # per-window [node, head] attention-dst tables, SBUF-resident
            tdT1 = cpool.tile([128, NWIN * 8], dt.bfloat16, name="tdT1")
            tdT2 = cpool.tile([128, NWIN * 8], dt.bfloat16, name="tdT2")
            nc.scalar.memzero(tdT1[:])
            nc.scalar.memzero(tdT2[:])
            outsb = cpool.tile([128, NWIN * 20], dt.float32, name="outsb")
            nc.scalar.memzero(outsb[:])

            # ---------- Phase A: T1 rows for local nodes ----------
            PHB = 4  # windows per xT load
            xt = None
            for w in range(NWIN):
                nw = min(WIN, NLOC - w * WIN)
                if w % PHB == 0:
                    ncols = min(PHB * WIN, NLOC - w * WIN)
                    xt = sb.tile([128, PHB * WIN], dt.float32r,
                                 name=f"xt{w}", tag="xt", bufs=2)
                    nc.sync.dma_start(out=xt[:, :ncols],
                                      in_=xT[:, w * WIN:w * WIN + ncols])
                xo = (w % PHB) * WIN
                pA = ps.tile([WIN, 256], dt.float32, name=f"pA{w}", tag="pA")
                nc.tensor.matmul(out=pA[:nw, 0:80], lhsT=xt[:, xo:xo + nw],
                                 rhs=w1t[:, 0:80], start=True, stop=True)
                t1 = sb.tile([WIN, 80], dt.bfloat16, name=f"t1o{w}", tag="t1o", bufs=3)
                nc.vector.tensor_copy(out=t1[:nw, :], in_=pA[:nw, 0:80])
                nc.sync.dma_start(out=ts1_loc[w * WIN:w * WIN + nw, 0:72],
                                  in_=t1[:nw, 0:72])
                nc.vector.tensor_copy(out=tdT1[:nw, w * 8:w * 8 + 8],
                                      in_=t1[:nw, 72:80])

            if not ABL_NO_COLL:
                for s in range(NBUCK):
                    nc.gpsimd.collective_compute(
                        "AllGather", mybir.AluOpType.bypass,
                        replica_groups=[list(range(R))],
                        ins=[ts1_loc[s * SLICE_H:(s + 1) * SLICE_H]],
                        outs=[ts1_full[s * SLAB:(s + 1) * SLAB]])

            # ---------- window epilogues ----------
            def epilogue1(wi, nw, pagg):
                # channel-major (e, h) layout throughout
                t8 = sb.tile([WIN, 8], dt.float32, name=f"t8a{wi}", tag="t8a")
                nc.vector.tensor_scalar(t8[:nw], pagg[:nw, 64:72], EPS, None,
                                        mybir.AluOpType.add)
                inv8 = sb.tile([WIN, 8], dt.float32, name=f"i8a{wi}", tag="i8a")
                nc.vector.reciprocal(inv8[:nw], t8[:nw])
                z = sb.tile([WIN, 64], dt.float32, name=f"z{wi}", tag="z")
                nc.vector.tensor_tensor(
                    out=z[:nw].rearrange("p (e h) -> p e h", h=8),
                    in0=pagg[:nw, 0:64].rearrange("p (e h) -> p e h", h=8),
                    in1=inv8[:nw].unsqueeze(1).broadcast_to([nw, 8, 8]),
                    op=mybir.AluOpType.mult)
                nc.vector.tensor_tensor(out=z[:nw], in0=z[:nw], in1=b1t[:nw],
                                        op=mybir.AluOpType.add)
                mz = sb.tile([WIN, 64], dt.float32, name=f"mz{wi}", tag="mz")
                nc.vector.tensor_scalar(mz[:nw], z[:nw], 0.0, None,
                                        mybir.AluOpType.min)
                nc.scalar.activation(mz[:nw], mz[:nw],
                                     mybir.ActivationFunctionType.Exp)
                rz = sb.tile([WIN, 64], dt.float32, name=f"rz{wi}", tag="rz")
                nc.scalar.activation(rz[:nw], z[:nw],
                                     mybir.ActivationFunctionType.Relu)
                nc.vector.tensor_tensor(out=z[:nw], in0=mz[:nw], in1=rz[:nw],
                                        op=mybir.AluOpType.add)
                h2 = sb.tile([WIN, 64], dt.float32, name=f"h2{wi}", tag="h2")
                nc.vector.tensor_scalar(h2[:nw], z[:nw], -1.0, None,
                                        mybir.AluOpType.add)
                ptr = ps.tile([64, WIN], dt.float32, name=f"ptr{wi}", tag="ptr")
                nc.tensor.transpose(out=ptr[:, :nw], in_=h2[:nw, :], identity=idt[:nw, :nw])
                h2T = sb.tile([64, WIN], dt.float32r, name=f"h2T{wi}", tag="h2T")
                nc.scalar.copy(out=h2T[:, :nw], in_=ptr[:, :nw])
                pT2 = ps.tile([WIN, 256], dt.float32, name=f"pT2{wi}", tag="pA")
                nc.tensor.matmul(out=pT2[:nw, 0:176], lhsT=h2T[:, :nw],
                                 rhs=w2t[:, 0:176], start=True, stop=True)
                t2o = sb.tile([WIN, 176], dt.bfloat16, name=f"t2o{wi}", tag="t2o", bufs=3)
                nc.vector.tensor_copy(out=t2o[:nw, :], in_=pT2[:nw, 0:176])
                nc.sync.dma_start(out=ts2_loc[wi * WIN:wi * WIN + nw, 0:168],
                                  in_=t2o[:nw, 0:168])
                nc.vector.tensor_copy(out=tdT2[:nw, wi * 8:wi * 8 + 8],
                                      in_=t2o[:nw, 168:176])

            def epilogue2(wi, nw, pagg):
                t8 = sb.tile([WIN, 8], dt.float32, name=f"t8b{wi}", tag="t8b")
                nc.vector.tensor_scalar(t8[:nw], pagg[:nw, 160:168], EPS, 8.0,
                                        mybir.AluOpType.add, mybir.AluOpType.mult)
                inv8 = sb.tile([WIN, 8], dt.float32, name=f"i8b{wi}", tag="i8b")
                nc.vector.reciprocal(inv8[:nw], t8[:nw])
                v = sb.tile([WIN, 20], dt.float32, name=f"v{wi}", tag="v")
                val = sb.tile([WIN, 160], dt.float32, name=f"val{wi}", tag="val")
                nc.vector.tensor_tensor(
                    out=val[:nw].rearrange("p (c h) -> p c h", h=8),
                    in0=pagg[:nw, 0:160].rearrange("p (c h) -> p c h", h=8),
                    in1=inv8[:nw].unsqueeze(1).broadcast_to([nw, 20, 8]),
                    op=mybir.AluOpType.mult)
                nc.vector.tensor_reduce(
                    out=v[:nw], in_=val[:nw].rearrange("p (c h) -> p c h", h=8),
                    axis=mybir.AxisListType.X, op=mybir.AluOpType.add)
                nc.vector.tensor_tensor(out=v[:nw], in0=v[:nw], in1=b2t[:nw],
                                        op=mybir.AluOpType.add)
                mx = sb.tile([WIN, 1], dt.float32, name=f"mx{wi}", tag="mx")
                nc.vector.tensor_reduce(out=mx[:nw], in_=v[:nw],
                                        axis=mybir.AxisListType.X,
                                        op=mybir.AluOpType.max)
                nmx = sb.tile([WIN, 1], dt.float32, name=f"nmx{wi}", tag="nmx")
                nc.vector.tensor_scalar(nmx[:nw], mx[:nw], -1.0, None,
                                        mybir.AluOpType.mult)
                ex = sb.tile([WIN, 20], dt.float32, name=f"exo{wi}", tag="exo")
                ssum = sb.tile([WIN, 1], dt.float32, name=f"ss{wi}", tag="ss")
                nc.scalar.activation(ex[:nw], v[:nw],
                                     mybir.ActivationFunctionType.Exp,
                                     bias=nmx[:nw], accum_out=ssum[:nw])
                lse = sb.tile([WIN, 1], dt.float32, name=f"lse{wi}", tag="lse")
                nc.scalar.activation(lse[:nw], ssum[:nw],
                                     mybir.ActivationFunctionType.Ln)
                nc.vector.tensor_tensor(
                    out=ex[:nw], in0=v[:nw],
                    in1=nmx[:nw].broadcast_to([nw, 20]),
                    op=mybir.AluOpType.add)
                nc.vector.tensor_tensor(
                    out=outsb[:nw, wi * 20:wi * 20 + 20], in0=ex[:nw],
                    in1=lse[:nw].broadcast_to([nw, 20]),
                    op=mybir.AluOpType.subtract)

            # ---------- slot init: zero gbuf pool slots so pad lanes read
            # finite stale data ----------
            if ZINIT:
                for ii in range(GBUFS):
                    gz = gb.tile([128, NCHG, 256], dt.bfloat16,
                                 name=f"gz{ii}", tag="gbuf", bufs=GBUFS)
                    nc.scalar.memzero(gz[:])

            # ---------- edge phase ----------
            def edge_phase(layer, slice_done=None):
                # last window whose epilogue completes collective slice s
                slice_last_w = [(min((s + 1) * SLICE_H, NLOC) - 1) // WIN
                                for s in range(NBUCK)]
                if layer == 1:
                    tsrc_full, tdT = ts1_full, tdT1
                    EW, CY, CH = 128, 72, 64
                else:
                    tsrc_full, tdT = ts2_full, tdT2
                    EW, CY, CH = 256, 168, 160

                qn = 0
                for (w0, nwg, gc0, nch_g, calls, win_pos, runs) in ginfo:
                    gbuf = gb.tile([128, nch_g, EW], dt.bfloat16,
                                   name=f"gbuf{layer}_{w0}", tag="gbuf", bufs=GBUFS,
                                   padded_shape=[128, NCHG * (256 // EW), EW])
                    pidxg = sb.tile([128, nch_g * 8], dt.int16,
                                    name=f"pi{layer}_{w0}", tag="pidx",
                                    padded_shape=[128, NCHG * 8])
                    nc.sync.dma_start(out=pidxg[:],
                                      in_=pay_idx[:, gc0 * 8:(gc0 + nch_g) * 8])
                    if not ABL_NO_PAY:
                        for (c0, c1, b) in calls:
                            k = c1 - c0
                            nc.gpsimd.dma_gather(
                                gbuf[:, c0:c1, :],
                                tsrc_full[b * SLAB:(b + 1) * SLAB, :],
                                pidxg[:, c0 * 8:c1 * 8],
                                k * 128, k * 128, EW,
                                queue_num=qn % 4, single_packet=True)
                            qn += 1
                    # dst-side attention: ohT [dst(part), chunk, lane],
                    # built per bucket-run to bound SBUF
                    eadst = ps.tile([128, nch_g * 8], dt.float32,
                                    name=f"ea{layer}_{w0}", tag="eadst",
                                    padded_shape=[128, NCHG * 8])
                    for (c0, c1, b) in runs:
                        rl = c1 - c0
                        dfTg = sb.tile([128, rl, 128], dt.bfloat16,
                                       name=f"dfT{layer}_{w0}_{b}", tag="dfT",
                                       padded_shape=[128, RUNMAX, 128])
                        nc.sync.dma_start(
                            out=dfTg[:],
                            in_=doffT_in[0:1, (gc0 + c0) * 128:(gc0 + c1) * 128]
                                .broadcast_to([128, rl * 128])
                                .rearrange("p (c l) -> p c l", l=128))
                        ohTg = sb.tile([128, rl, 128], dt.bfloat16,
                                       name=f"ohT{layer}_{w0}_{b}", tag="ohT",
                                       padded_shape=[128, RUNMAX, 128])
                        nc.vector.tensor_scalar(ohTg[:], dfTg[:], iotP[:, 0:1],
                                                None, mybir.AluOpType.is_equal)
                        for ci in range(c0, c1):
                            wi = chunk_window[gc0 + ci]
                            nc.tensor.matmul(
                                out=eadst[:, ci * 8:ci * 8 + 8],
                                lhsT=ohTg[:, ci - c0, :],
                                rhs=tdT[:, wi * 8:wi * 8 + 8],
                                start=True, stop=True)
                    ebuf = sb.tile([128, nch_g, 8], dt.float32,
                                   name=f"eb{layer}_{w0}", tag="ebuf",
                                   padded_shape=[128, NCHG, 8])
                    nc.vector.tensor_tensor(
                        out=ebuf[:], in0=gbuf[:, :, CH:CH + 8],
                        in1=eadst[:].rearrange("p (c h) -> p c h", h=8),
                        op=mybir.AluOpType.add)
                    nc.scalar.activation(ebuf[:], ebuf[:],
                                         mybir.ActivationFunctionType.Prelu,
                                         alpha=NEG_SLOPE)
                    nc.scalar.activation(gbuf[:, :, CH:CH + 8], ebuf[:],
                                         mybir.ActivationFunctionType.Exp)
                    # channel-major payload: (e-or-c, h) with h innermost
                    ne = CH // 8
                    nc.vector.tensor_tensor(
                        out=gbuf[:, :, 0:CH].rearrange(
                            "p c (e h) -> p c e h", h=8),
                        in0=gbuf[:, :, 0:CH].rearrange(
                            "p c (e h) -> p c e h", h=8),
                        in1=gbuf[:, :, CH:CH + 8].unsqueeze(2)
                            .broadcast_to([128, nch_g, ne, 8]),
                        op=mybir.AluOpType.mult)
                    # aggregation one-hot oh [lane(part), dstoff, chunk]
                    ohg = sb.tile([128, 128, nch_g], dt.bfloat16,
                                  name=f"oh{layer}_{w0}", tag="oh",
                                  padded_shape=[128, 128, NCHG])
                    nc.vector.tensor_tensor(
                        out=ohg[:],
                        in0=dofft[:, gc0:gc0 + nch_g].unsqueeze(1)
                            .broadcast_to([128, 128, nch_g]),
                        in1=iotc[:, :, :nch_g],
                        op=mybir.AluOpType.is_equal)
                    for wi in range(w0, w0 + nwg):
                        nw_nodes = min(WIN, NLOC - wi * WIN)
                        pos = win_pos[wi]
                        pagg = ps.tile([WIN, CY], dt.float32,
                                       name=f"pg{layer}_{wi}", tag="pagg",
                                       padded_shape=[WIN, 168])
                        nmm = 1 if ABL_NO_AGG else len(pos)
                        for j in range(nmm):
                            ci = pos[j]
                            nc.tensor.matmul(
                                out=pagg[:, :],
                                lhsT=ohg[:, :, ci],
                                rhs=gbuf[:, ci, 0:CY],
                                start=(j == 0), stop=(j == nmm - 1))
                        if layer == 1:
                            epilogue1(wi, nw_nodes, pagg)
                        else:
                            epilogue2(wi, nw_nodes, pagg)
                        if slice_done is not None and wi in slice_last_w:
                            slice_done(slice_last_w.index(wi))

            def coll2_slice(s):
                if not ABL_NO_COLL:
                    nc.gpsimd.collective_compute(
                        "AllGather", mybir.AluOpType.bypass,
                        replica_groups=[list(range(R))],
                        ins=[ts2_loc[s * SLICE_H:(s + 1) * SLICE_H]],
                        outs=[ts2_full[s * SLAB:(s + 1) * SLAB]])

            edge_phase(1, coll2_slice)
            edge_phase(2)
            nc.sync.dma_start(out=out_ext[:], in_=outsb[:])

    orig = bacc.get_activation_tables
    bacc.get_activation_tables = _unified_act_tables(orig)
    try:
        nc.compile()
    finally:
        bacc.get_activation_tables = orig
    return nc


def _wext(w, a_src, a_dst, out_rows, pad_cols=256, heads=8):
    """[W(ch-major) | W.a_src | W.a_dst] padded to [out_rows, pad_cols] f32.
    Output feature columns are channel-major: col = e*H + h."""
    f = w.shape[0]
    c = w.shape[1] // heads
    w3 = w.reshape(f, heads, c)
    wmain = w3.transpose(0, 2, 1).reshape(f, heads * c)  # col = e*H + h
    was = np.einsum("fhc,hc->fh", w3, a_src)
    wad = np.einsum("fhc,hc->fh", w3, a_dst)
    out = np.zeros((out_rows, pad_cols), dtype=np.float32)
    out[:f, :w.shape[1]] = wmain
    out[:f, w.shape[1]:w.shape[1] + heads] = was
    out[:f, w.shape[1] + heads:w.shape[1] + 2 * heads] = wad
    return out


_CACHE = {}


def kernel(x, edge_index, w1, att_src1, att_dst1, b1, w2, att_src2, att_dst2, b2):
    x = np.asarray(x, dtype=np.float32)
    edge_index = np.asarray(edge_index)
    src = np.concatenate([edge_index[0], np.arange(N_NODES, dtype=np.int64)]).astype(np.int64)
    dst = np.concatenate([edge_index[1], np.arange(N_NODES, dtype=np.int64)]).astype(np.int64)

    key = hash(edge_index.tobytes())
    if key not in _CACHE:
        lay = _build_layout(src, dst)
        nkern = build_kernel(lay)
        _CACHE[key] = (lay, nkern)
    lay, nkern = _CACHE[key]

    in_maps = _prep_in_maps(x, w1, att_src1, att_dst1, b1,
                            w2, att_src2, att_dst2, b2, lay)
    res = run_bass_kernel_spmd(nkern, in_maps, core_ids=list(range(R)))
    parts = []
    for r in range(R):
        o = np.asarray(res.results[r]["out"])  # [128, NWIN*20] partition-major
        o = o.reshape(128, NWIN, 20).transpose(1, 0, 2).reshape(NWIN * 128, 20)
        parts.append(o[:NLOC])
    return np.concatenate(parts, axis=0).astype(np.float32)


def _prep_in_maps(x, w1, att_src1, att_dst1, b1, w2, att_src2, att_dst2, b2, lay):
    NCHG = lay["NCHG"]
    cores = lay["cores"]
    H = 8
    # layer-1 channel-major: h1 columns (e,h); b1 likewise
    w1p = _wext(np.asarray(w1, np.float32), np.asarray(att_src1, np.float32),
                np.asarray(att_dst1, np.float32), 128)
    # layer-2 input rows follow layer-1 ch-major order: permute rows of w2
    w2a = np.asarray(w2, np.float32)
    rowperm = (np.arange(64).reshape(8, 8).T.reshape(64))  # new r=(e*8+h) -> old h*8+e
    w2perm = w2a[rowperm]
    w2p = _wext(w2perm, np.asarray(att_src2, np.float32),
                np.asarray(att_dst2, np.float32), 64)
    b1cm = np.asarray(b1, np.float32).reshape(H, 8).T.reshape(64)
    b1rep = np.tile(b1cm[None, :], (128, 1))
    b2rep = np.tile(np.asarray(b2, np.float32)[None, :], (128, 1))
    ident = np.eye(128, dtype=np.float32)
    iotc = np.broadcast_to(np.arange(128, dtype=np.float32)[:, None],
                           (128, NCHG)).astype(BF16)
    iotc = np.broadcast_to(iotc.reshape(1, 128, NCHG), (128, 128, NCHG))
    iotc = np.ascontiguousarray(iotc.reshape(128, 128 * NCHG))
    iotP = np.arange(128, dtype=np.float32).reshape(128, 1)
    in_maps = []
    for r in range(R):
        xTr = np.ascontiguousarray(x[r * NLOC:(r + 1) * NLOC].T)
        in_maps.append({
            "xT": xTr, "w1p": w1p, "w2p": w2p, "b1rep": b1rep, "b2rep": b2rep,
            "ident": ident,
            "doff": cores[r]["dstoff"], "doffT": cores[r]["dstoffT"],
            "iotc": iotc, "iotP": iotP, "pay_idx": cores[r]["pay_idx"],
        })
    return in_maps
